# revision 13
# baseline (speedup 1.0000x reference)
"""Trainium2 Bass kernel for NeuralFractionalDE.

out = x_current + drift(x)*DT + softplus_head(x)*(noise*DT^H) + frac_deriv*(ALPHA*DT)

where frac_deriv = sum_k (x_hist[:,k+1,:]-x_hist[:,k,:]) * w[k] collapses to
sum_t c[t] * x_hist[:,t,:] with c[t] = w[t-1]-w[t] (boundary adjusted).

Data parallel over 8 NeuronCores (256 batch rows each). The x_history
stream is cast to fp8 e4m3 on the host (error contribution ~5e-5 rel-fro,
two orders under the gate) and rearranged host-side to a partition-major
layout: t = 8*p + ti, so each partition's whole stream is contiguous in
HBM (one 8 KiB descriptor per partition per group). The time reduction
runs on the TensorEngine as DoubleRow fp8 matmuls: each instruction
contracts 2 timesteps (128 partitions x 2), 4 accumulating matmuls per
512-wide PSUM row. Coefficients are scaled by 64 into fp8 range; the
ALPHA*DT/64 factor is applied in the PSUM copy-out.

Output assembly: base rows (xc + drift*DT + diffusion*fbm) are written to
DRAM `out` once, then the stream results are CCE-accumulated into `out`
by SWDGE scatter DMAs, batched 4 groups per descriptor chain (per-group
4 KiB accum DMAs serialize at ~2.8us each on the Q7 + completion path
and would lag the stream). All fp32 constants arrive via one packed
[128, NCOL] tensor: per-tensor const loads cost ~0.7us of HWDGE ring
dispatch each and delayed the MLP by ~20us.
"""

import math

import numpy as np

try:
    import concourse.bass as bass
except ImportError:  # pragma: no cover
    import sys

    sys.path.insert(0, "/opt/trn_rl_repo")
    import concourse.bass as bass

import ml_dtypes

import concourse.bacc as bacc
import concourse.mybir as mybir
import concourse.tile as tile
from concourse.bass_utils import run_bass_kernel_spmd

ALPHA = 0.7
K = 1024
DT = 0.01
H = 0.5 + ALPHA / 2
D = 128
HID = 256
B = 2048
N_CORES = 8
B_PER = B // N_CORES  # 256
TI = 8  # time sub-steps per partition: t = TI*p + ti
NB = 8  # batch rows per group
G = B_PER // NB  # 32 groups; batch b = NB*g + bi
SB = 4  # groups per scatter-accum batch
CSCALE = 64.0  # fp8 range scale for the frac coefficients
SCL_OUT = float(ALPHA * DT / CSCALE)

F32 = mybir.dt.float32
FP8 = mybir.dt.float8e4
AF = mybir.ActivationFunctionType
OP = mybir.AluOpType
E4M3 = ml_dtypes.float8_e4m3


def _pack_spec():
    cols = {}
    off = 0

    def add(name, w):
        nonlocal off
        cols[name] = (off, w)
        off += w

    add("ident", 128)
    add("xc0", 128)
    add("xc1", 128)
    add("nz0", 1)
    add("nz1", 1)
    for net in ("d", "g"):
        add(net + "w1", HID)
        add(net + "w2_0", HID)
        add(net + "w2_1", HID)
        add(net + "w3_0", D)
        add(net + "w3_1", D)
        add(net + "b1_0", 1)
        add(net + "b1_1", 1)
        add(net + "b2_0", 1)
        add(net + "b2_1", 1)
        add(net + "b3", 1)
    return cols, off


COLS, NCOL = _pack_spec()


def _coeffs_fp8() -> np.ndarray:
    """c8st[p, ti, 0] = c[TI*p + ti] * CSCALE in fp8; Ko-stride 16 B."""
    t = np.arange(1, K + 1, dtype=np.float64)
    kern = (t ** (-ALPHA)) / math.gamma(1.0 - ALPHA)
    w = kern[::-1][: K - 1]  # w[k] = kern[K-1-k]
    c = np.zeros(K, dtype=np.float64)
    c[1:] += w
    c[: K - 1] -= w
    c *= CSCALE
    arr = np.zeros((128, TI, 16), dtype=E4M3)
    arr[:, :, 0] = c.reshape(128, TI).astype(np.float32).astype(E4M3)
    return arr


def _build_program() -> bass.Bass:
    # Bacc (not raw Bass): its compile() legalizes semaphore waits to the
    # 1-wait-per-instruction ISA limit (generate_event_semaphores).
    nc = bacc.Bacc(None, target_bir_lowering=False)

    xh = nc.dram_tensor("xh", [128, G, TI, NB, D], FP8, kind="ExternalInput")
    wp = nc.dram_tensor("wp", [128, NCOL], F32, kind="ExternalInput")
    out = nc.dram_tensor("out", [B_PER, D], F32, kind="ExternalOutput")

    c8d = nc.inline_tensor(_coeffs_fp8(), name="c8const")

    with tile.TileContext(nc) as tc:
        with (
            tc.tile_pool(name="const", bufs=1) as cpool,
            tc.tile_pool(name="stream", bufs=8) as spool,
            tc.tile_pool(name="stg", bufs=4) as gpool,
            tc.tile_pool(name="psf", bufs=3, space=bass.MemorySpace.PSUM) as psf,
            tc.tile_pool(name="psm", bufs=2, space=bass.MemorySpace.PSUM) as psm,
            tc.tile_pool(name="pst", bufs=3, space=bass.MemorySpace.PSUM) as pst,
        ):
            # ---- constants: one fp8 coeff load + one packed fp32 load on
            # the scalar (ACT) HWDGE ring; the sync ring is reserved for
            # the x_history stream ----
            c8_sb = cpool.tile([128, TI, 16], FP8, tag="c8")
            nc.scalar.dma_start(out=c8_sb[:], in_=c8d[:])
            wp_sb = cpool.tile([128, NCOL], F32, tag="wp")
            nc.scalar.dma_start(out=wp_sb[:], in_=wp[:])

            def C(nm, c0=0, w=None):
                off, width = COLS[nm]
                if w is None:
                    w = width
                return wp_sb[:, off + c0 : off + c0 + w]

            base_sb = [
                cpool.tile([128, D], F32, tag=f"base{tb}", name=f"base{tb}")
                for tb in range(2)
            ]
            xcT_sb = cpool.tile([128, B_PER], F32, tag="xcT")
            driftT_sb = cpool.tile([128, B_PER], F32, tag="driftT")
            diffT_sb = cpool.tile([128, B_PER], F32, tag="diffT")

            # ACT LUT discipline: tanh and exp live in the same table set
            # (exp_and_others), ln in another; each ACT_TABLE_LOAD costs
            # ~1.3us on the ACT queue that also drains stream PSUMs. Using
            # AF.Tanh directly + one Exp->Ln switch for the softplus head
            # keeps it to a single mid-kernel table load. (Copy is a
            # size-1 "others" entry present in every table.)

            h_sb = {}  # MLP hidden tiles, created per stage

            # ---- MLP emitted piecewise between stream groups so the PE
            # queue never stalls on ACT/DVE latency ----
            def mlp_stage_xcT():
                for tb in range(2):
                    pt = pst.tile([128, 128], F32, tag="pst")
                    nc.tensor.transpose(pt[:], C(f"xc{tb}"), C("ident"))
                    nc.scalar.activation(
                        xcT_sb[:, tb * 128 : (tb + 1) * 128], pt[:], AF.Copy
                    )

            def mlp_stage_h1():
                for net in ("d", "g"):
                    h1 = []
                    for j in range(2):
                        ps = psm.tile([128, B_PER], F32, tag="psm")
                        nc.tensor.matmul(
                            ps[:],
                            C(net + "w1", j * 128, 128),
                            xcT_sb[:],
                            start=True,
                            stop=True,
                        )
                        h = cpool.tile([128, B_PER], F32, tag=f"{net}h1{j}")
                        nc.scalar.activation(
                            h[:], ps[:], AF.Tanh, bias=C(net + f"b1_{j}")
                        )
                        h1.append(h)
                    h_sb[net + "h1"] = h1

            def mlp_stage_h2():
                for net in ("d", "g"):
                    h1 = h_sb[net + "h1"]
                    h2 = []
                    for j in range(2):
                        ps = psm.tile([128, B_PER], F32, tag="psm")
                        for i in range(2):
                            nc.tensor.matmul(
                                ps[:],
                                C(net + f"w2_{i}", j * 128, 128),
                                h1[i][:],
                                start=(i == 0),
                                stop=(i == 1),
                            )
                        h = cpool.tile([128, B_PER], F32, tag=f"{net}h2{j}")
                        nc.scalar.activation(
                            h[:], ps[:], AF.Tanh, bias=C(net + f"b2_{j}")
                        )
                        h2.append(h)
                    h_sb[net + "h2"] = h2

            def mlp_stage_out():
                for net in ("d", "g"):
                    h2 = h_sb[net + "h2"]
                    ps = psm.tile([128, B_PER], F32, tag="psm")
                    for i in range(2):
                        nc.tensor.matmul(
                            ps[:],
                            C(net + f"w3_{i}"),
                            h2[i][:],
                            start=(i == 0),
                            stop=(i == 1),
                        )
                    if net == "d":
                        # driftT = (raw + b3) * DT
                        nc.vector.tensor_scalar(
                            out=driftT_sb[:],
                            in0=ps[:],
                            scalar1=C("db3"),
                            scalar2=float(DT),
                            op0=OP.add,
                            op1=OP.mult,
                        )
                    else:
                        # softplus via ln(1 + exp(x + b)); the Ln is the
                        # kernel's single ACT table switch
                        nc.scalar.activation(
                            diffT_sb[:], ps[:], AF.Exp, bias=C("gb3")
                        )
                        nc.scalar.activation(diffT_sb[:], diffT_sb[:], AF.Ln, bias=1.0)

            def mlp_stage_base():
                # base[tb] = xc + driftT^T (already *DT) + diffT^T*nz*DT^H
                for tb in range(2):
                    ptd = pst.tile([128, 128], F32, tag="pst")
                    nc.tensor.transpose(
                        ptd[:], driftT_sb[:, tb * 128 : (tb + 1) * 128], C("ident")
                    )
                    ptg = pst.tile([128, 128], F32, tag="pst")
                    nc.tensor.transpose(
                        ptg[:], diffT_sb[:, tb * 128 : (tb + 1) * 128], C("ident")
                    )
                    b_ = base_sb[tb]
                    # base = diffusion * noise * DT^H
                    nc.vector.tensor_scalar(
                        out=b_[:],
                        in0=ptg[:],
                        scalar1=C(f"nz{tb}"),
                        scalar2=float(DT**H),
                        op0=OP.mult,
                        op1=OP.mult,
                    )
                    nc.vector.tensor_add(out=b_[:], in0=b_[:], in1=ptd[:])
                    nc.vector.tensor_add(out=b_[:], in0=b_[:], in1=C(f"xc{tb}"))

            mlp_stages = {
                1: mlp_stage_xcT,
                2: mlp_stage_h1,
                3: mlp_stage_h2,
                4: mlp_stage_out,
                5: mlp_stage_base,
            }

            # ---- fractional-derivative stream: the 32 MiB fp8 scan ----
            # xh[p, g, ti, bi, d]: per partition, one 8 KiB contiguous read
            # per group. DoubleRow contracts timestep pairs (2u, 2u+1):
            # lhsT = c8[:, 2u:2u+2, 0:1] (Ko stride 16 B), rhs free = 1024
            # -> psum [1, 512] over 4 accumulating matmuls.
            DR = mybir.MatmulPerfMode.DoubleRow
            # out rows NB*SB*q .. +NB*SB are contiguous in DRAM and match
            # the stage batch's element order exactly: one accum DMA per
            # SB groups.
            out_flat = out.rearrange("(q x) d -> q (x d)", x=NB * SB)

            def scatter_accum(q, stage4):
                nc.gpsimd.dma_start(
                    out=out_flat[q : q + 1],
                    in_=stage4[0:1],
                    accum_op=OP.add,
                )

            stage4 = None
            pending = []  # (q, stage4) scatters held until base rows land
            for g in range(G):
                xt = spool.tile([128, TI, NB, D], FP8, tag="xt")
                # alternate the stream between the two HWDGE rings (SP and
                # ACT): while one ring's tile drains, the other's
                # descriptors are already doorbelled, hiding the per-DMA
                # dispatch/doorbell bubble (~0.25us/tile on one ring)
                ring = nc.sync if g % 2 == 0 else nc.scalar
                if g < 2:
                    # quarter loads so the first matmul starts ~3.5us in
                    for u in range(TI // 2):
                        ring.dma_start(
                            out=xt[:, 2 * u : 2 * u + 2], in_=xh[:, g, 2 * u : 2 * u + 2]
                        )
                else:
                    ring.dma_start(out=xt[:], in_=xh[:, g])
                if g % SB == 0:
                    stage4 = gpool.tile([1, SB * NB * D], F32, tag="stage")
                soff = (g % SB) * NB * D
                for cb in range(2):
                    ps = psf.tile([1, 512], F32, tag="psf")
                    for u in range(TI // 2):
                        nc.tensor.matmul(
                            ps[:],
                            c8_sb[:, 2 * u : 2 * u + 2, 0:1],
                            xt[:, 2 * u : 2 * u + 2, 4 * cb : 4 * cb + 4, :],
                            start=(u == 0),
                            stop=(u == TI // 2 - 1),
                            perf_mode=DR,
                        )
                    # drains stay off the ACT queue so the scalar ring's
                    # stream dispatches never wait behind ACT compute
                    nc.vector.tensor_scalar(
                        out=stage4[0:1, soff + cb * 512 : soff + (cb + 1) * 512],
                        in0=ps[:],
                        scalar1=SCL_OUT,
                        scalar2=None,
                        op0=OP.mult,
                    )
                if g in mlp_stages:
                    mlp_stages[g]()
                if g == 5:
                    # base rows -> out, then release the held scatter
                    for tb in range(2):
                        nc.scalar.dma_start(
                            out=out[tb * 128 : (tb + 1) * 128, :],
                            in_=base_sb[tb][:],
                        )
                    for qp, sp in pending:
                        scatter_accum(qp, sp)
                    pending.clear()
                if g % SB == SB - 1:
                    q = g // SB
                    if g < 5:
                        pending.append((q, stage4))
                    else:
                        scatter_accum(q, stage4)

    nc.compile()
    return nc


_NC_CACHE = None


def _get_program() -> bass.Bass:
    global _NC_CACHE
    if _NC_CACHE is None:
        _NC_CACHE = _build_program()
    return _NC_CACHE


def _pack_consts(inputs: dict, xc: np.ndarray, nz: np.ndarray, core: int) -> np.ndarray:
    pk = np.zeros((128, NCOL), dtype=np.float32)

    def put(nm, arr):
        off, w = COLS[nm]
        pk[:, off : off + w] = arr.reshape(128, w)

    s = slice(core * B_PER, (core + 1) * B_PER)
    xcc, nzc = xc[s], nz[s]
    put("ident", np.eye(128, dtype=np.float32))
    put("xc0", xcc[0:128])
    put("xc1", xcc[128:256])
    put("nz0", nzc[0:128])
    put("nz1", nzc[128:256])
    for net in ("d", "g"):
        put(net + "w1", inputs[net + "w1"])
        w2 = inputs[net + "w2"]
        put(net + "w2_0", w2[0:128])
        put(net + "w2_1", w2[128:256])
        w3 = inputs[net + "w3"]
        put(net + "w3_0", w3[0:128])
        put(net + "w3_1", w3[128:256])
        b1 = inputs[net + "b1"]
        put(net + "b1_0", b1[0:128])
        put(net + "b1_1", b1[128:256])
        b2 = inputs[net + "b2"]
        put(net + "b2_0", b2[0:128])
        put(net + "b2_1", b2[128:256])
        put(net + "b3", inputs[net + "b3"])
    return pk


def _in_maps(inputs: dict) -> list[dict]:
    f = lambda x: np.ascontiguousarray(np.asarray(x, dtype=np.float32))
    xh = np.asarray(inputs["x_history"], dtype=np.float32)
    xc = f(inputs["x_current"])
    nz = f(inputs["noise"])
    assert xh.shape == (B, K, D) and xc.shape == (B, D) and nz.shape == (B,)
    # [core, g, bi, p, ti, d] -> [core, p, g, ti, bi, d], cast to fp8 e4m3
    xh8 = (
        xh.reshape(N_CORES, G, NB, 128, TI, D)
        .transpose(0, 3, 1, 4, 2, 5)
        .astype(E4M3)
    )
    ws = {k: f(inputs[k]) for k in inputs if k[0] in "dg" and k != "noise"}
    maps = []
    for c in range(N_CORES):
        maps.append({"xh": xh8[c], "wp": _pack_consts(ws, xc, nz, c)})
    return maps


def run(inputs: dict, trace: bool = False):
    nc = _get_program()
    res = run_bass_kernel_spmd(nc, _in_maps(inputs), list(range(N_CORES)), trace=trace)
    out = np.concatenate([res.results[c]["out"] for c in range(N_CORES)], axis=0)
    return out, res


def kernel(**inputs) -> np.ndarray:
    out, _ = run(inputs, trace=False)
    return out


# revision 18
# speedup vs baseline: 1.0167x; 1.0167x over previous
"""Trainium2 Bass kernel for NeuralFractionalDE.

out = x_current + drift(x)*DT + softplus_head(x)*(noise*DT^H) + frac_deriv*(ALPHA*DT)

where frac_deriv = sum_k (x_hist[:,k+1,:]-x_hist[:,k,:]) * w[k] collapses to
sum_t c[t] * x_hist[:,t,:] with c[t] = w[t-1]-w[t] (boundary adjusted).

Data parallel over 8 NeuronCores (256 batch rows each). The x_history
stream is cast to fp8 e4m3 on the host (error contribution ~5e-5 rel-fro,
two orders under the gate) and rearranged host-side to a partition-major
layout: t = 8*p + ti, so each partition's whole stream is contiguous in
HBM (one 8 KiB descriptor per partition per group). The time reduction
runs on the TensorEngine as DoubleRow fp8 matmuls: each instruction
contracts 2 timesteps (128 partitions x 2), 4 accumulating matmuls per
512-wide PSUM row. Coefficients are scaled by 64 into fp8 range; the
ALPHA*DT/64 factor is applied in the PSUM copy-out.

Output assembly: base rows (xc + drift*DT + diffusion*fbm) are written to
DRAM `out` once, then the stream results are CCE-accumulated into `out`
by SWDGE scatter DMAs, batched 4 groups per descriptor chain (per-group
4 KiB accum DMAs serialize at ~2.8us each on the Q7 + completion path
and would lag the stream). All fp32 constants arrive via one packed
[128, NCOL] tensor: per-tensor const loads cost ~0.7us of HWDGE ring
dispatch each and delayed the MLP by ~20us.
"""

import math

import numpy as np

try:
    import concourse.bass as bass
except ImportError:  # pragma: no cover
    import sys

    sys.path.insert(0, "/opt/trn_rl_repo")
    import concourse.bass as bass

import ml_dtypes

import concourse.bacc as bacc
import concourse.mybir as mybir
import concourse.tile as tile
from concourse.bass_utils import run_bass_kernel_spmd

ALPHA = 0.7
K = 1024
DT = 0.01
H = 0.5 + ALPHA / 2
D = 128
HID = 256
B = 2048
N_CORES = 8
B_PER = B // N_CORES  # 256
TI = 8  # time sub-steps per partition: t = TI*p + ti
NB = 8  # batch rows per group
G = B_PER // NB  # 32 groups; batch b = NB*g + bi
SB = 4  # groups per scatter-accum batch
CSCALE = 64.0  # fp8 range scale for the frac coefficients
SCL_OUT = float(ALPHA * DT / CSCALE)

F32 = mybir.dt.float32
FP8 = mybir.dt.float8e4
AF = mybir.ActivationFunctionType
OP = mybir.AluOpType
E4M3 = ml_dtypes.float8_e4m3


def _pack_spec():
    cols = {}
    off = 0

    def add(name, w):
        nonlocal off
        cols[name] = (off, w)
        off += w

    add("ident", 128)
    add("xc0", 128)
    add("xc1", 128)
    add("nz0", 1)
    add("nz1", 1)
    for net in ("d", "g"):
        add(net + "w1", HID)
        add(net + "w2_0", HID)
        add(net + "w2_1", HID)
        add(net + "w3_0", D)
        add(net + "w3_1", D)
        add(net + "b1_0", 1)
        add(net + "b1_1", 1)
        add(net + "b2_0", 1)
        add(net + "b2_1", 1)
        add(net + "b3", 1)
    return cols, off


COLS, NCOL = _pack_spec()


def _coeffs_fp8() -> np.ndarray:
    """c8st[p, ti, 0] = c[TI*p + ti] * CSCALE in fp8; Ko-stride 16 B."""
    t = np.arange(1, K + 1, dtype=np.float64)
    kern = (t ** (-ALPHA)) / math.gamma(1.0 - ALPHA)
    w = kern[::-1][: K - 1]  # w[k] = kern[K-1-k]
    c = np.zeros(K, dtype=np.float64)
    c[1:] += w
    c[: K - 1] -= w
    c *= CSCALE
    arr = np.zeros((128, TI, 16), dtype=E4M3)
    arr[:, :, 0] = c.reshape(128, TI).astype(np.float32).astype(E4M3)
    return arr


def _build_program() -> bass.Bass:
    # Bacc (not raw Bass): its compile() legalizes semaphore waits to the
    # 1-wait-per-instruction ISA limit (generate_event_semaphores).
    nc = bacc.Bacc(None, target_bir_lowering=False)

    xh = nc.dram_tensor("xh", [128, G, TI, NB, D], FP8, kind="ExternalInput")
    wp = nc.dram_tensor("wp", [128, NCOL], F32, kind="ExternalInput")
    out = nc.dram_tensor("out", [B_PER, D], F32, kind="ExternalOutput")

    c8d = nc.inline_tensor(_coeffs_fp8(), name="c8const")

    with tile.TileContext(nc) as tc:
        with (
            tc.tile_pool(name="const", bufs=1) as cpool,
            tc.tile_pool(name="stream", bufs=8) as spool,
            tc.tile_pool(name="stg", bufs=4) as gpool,
            tc.tile_pool(name="psf", bufs=4, space=bass.MemorySpace.PSUM) as psf,
            tc.tile_pool(name="psm", bufs=2, space=bass.MemorySpace.PSUM) as psm,
            tc.tile_pool(name="pst", bufs=2, space=bass.MemorySpace.PSUM) as pst,
        ):
            # ---- constants: one fp8 coeff load + one packed fp32 load on
            # the scalar (ACT) HWDGE ring; the sync ring is reserved for
            # the x_history stream ----
            c8_sb = cpool.tile([128, TI, 16], FP8, tag="c8")
            nc.scalar.dma_start(out=c8_sb[:], in_=c8d[:])
            wp_sb = cpool.tile([128, NCOL], F32, tag="wp")
            nc.scalar.dma_start(out=wp_sb[:], in_=wp[:])

            def C(nm, c0=0, w=None):
                off, width = COLS[nm]
                if w is None:
                    w = width
                return wp_sb[:, off + c0 : off + c0 + w]

            base_sb = [
                cpool.tile([128, D], F32, tag=f"base{tb}", name=f"base{tb}")
                for tb in range(2)
            ]
            xcT_sb = cpool.tile([128, B_PER], F32, tag="xcT")
            driftT_sb = cpool.tile([128, B_PER], F32, tag="driftT")
            diffT_sb = cpool.tile([128, B_PER], F32, tag="diffT")

            # ACT LUT discipline: tanh and exp live in the same table set
            # (exp_and_others), ln in another; each ACT_TABLE_LOAD costs
            # ~1.3us on the ACT queue that also drains stream PSUMs. Using
            # AF.Tanh directly + one Exp->Ln switch for the softplus head
            # keeps it to a single mid-kernel table load. (Copy is a
            # size-1 "others" entry present in every table.)

            h_sb = {}  # MLP hidden tiles, created per stage

            # ---- MLP emitted piecewise between stream groups so the PE
            # queue never stalls on ACT/DVE latency ----
            def mlp_stage_xcT():
                for tb in range(2):
                    pt = pst.tile([128, 128], F32, tag="pst")
                    nc.tensor.transpose(pt[:], C(f"xc{tb}"), C("ident"))
                    nc.scalar.activation(
                        xcT_sb[:, tb * 128 : (tb + 1) * 128], pt[:], AF.Copy
                    )

            def mlp_stage_h1():
                for net in ("d", "g"):
                    h1 = []
                    for j in range(2):
                        ps = psm.tile([128, B_PER], F32, tag="psm")
                        nc.tensor.matmul(
                            ps[:],
                            C(net + "w1", j * 128, 128),
                            xcT_sb[:],
                            start=True,
                            stop=True,
                        )
                        h = cpool.tile([128, B_PER], F32, tag=f"{net}h1{j}")
                        nc.scalar.activation(
                            h[:], ps[:], AF.Tanh, bias=C(net + f"b1_{j}")
                        )
                        h1.append(h)
                    h_sb[net + "h1"] = h1

            def mlp_stage_h2():
                for net in ("d", "g"):
                    h1 = h_sb[net + "h1"]
                    h2 = []
                    for j in range(2):
                        ps = psm.tile([128, B_PER], F32, tag="psm")
                        for i in range(2):
                            nc.tensor.matmul(
                                ps[:],
                                C(net + f"w2_{i}", j * 128, 128),
                                h1[i][:],
                                start=(i == 0),
                                stop=(i == 1),
                            )
                        h = cpool.tile([128, B_PER], F32, tag=f"{net}h2{j}")
                        nc.scalar.activation(
                            h[:], ps[:], AF.Tanh, bias=C(net + f"b2_{j}")
                        )
                        h2.append(h)
                    h_sb[net + "h2"] = h2

            def mlp_stage_out():
                for net in ("d", "g"):
                    h2 = h_sb[net + "h2"]
                    ps = psm.tile([128, B_PER], F32, tag="psm")
                    for i in range(2):
                        nc.tensor.matmul(
                            ps[:],
                            C(net + f"w3_{i}"),
                            h2[i][:],
                            start=(i == 0),
                            stop=(i == 1),
                        )
                    if net == "d":
                        # driftT = (raw + b3) * DT
                        nc.vector.tensor_scalar(
                            out=driftT_sb[:],
                            in0=ps[:],
                            scalar1=C("db3"),
                            scalar2=float(DT),
                            op0=OP.add,
                            op1=OP.mult,
                        )
                    else:
                        # softplus via ln(1 + exp(x + b)); the Ln is the
                        # kernel's single ACT table switch
                        nc.scalar.activation(
                            diffT_sb[:], ps[:], AF.Exp, bias=C("gb3")
                        )
                        nc.scalar.activation(diffT_sb[:], diffT_sb[:], AF.Ln, bias=1.0)

            def mlp_stage_base():
                # base[tb] = xc + driftT^T (already *DT) + diffT^T*nz*DT^H
                for tb in range(2):
                    ptd = pst.tile([128, 128], F32, tag="pst")
                    nc.tensor.transpose(
                        ptd[:], driftT_sb[:, tb * 128 : (tb + 1) * 128], C("ident")
                    )
                    ptg = pst.tile([128, 128], F32, tag="pst")
                    nc.tensor.transpose(
                        ptg[:], diffT_sb[:, tb * 128 : (tb + 1) * 128], C("ident")
                    )
                    b_ = base_sb[tb]
                    # base = diffusion * noise * DT^H
                    nc.vector.tensor_scalar(
                        out=b_[:],
                        in0=ptg[:],
                        scalar1=C(f"nz{tb}"),
                        scalar2=float(DT**H),
                        op0=OP.mult,
                        op1=OP.mult,
                    )
                    nc.vector.tensor_add(out=b_[:], in0=b_[:], in1=ptd[:])
                    nc.vector.tensor_add(out=b_[:], in0=b_[:], in1=C(f"xc{tb}"))

            mlp_stages = {
                1: mlp_stage_xcT,
                2: mlp_stage_h1,
                3: mlp_stage_h2,
                4: mlp_stage_out,
                5: mlp_stage_base,
            }

            # ---- fractional-derivative stream: the 32 MiB fp8 scan ----
            # xh[p, g, ti, bi, d]: per partition, one 8 KiB contiguous read
            # per group. DoubleRow contracts timestep pairs (2u, 2u+1):
            # lhsT = c8[:, 2u:2u+2, 0:1] (Ko stride 16 B), rhs free = 1024
            # -> psum [1, 512] over 4 accumulating matmuls.
            DR = mybir.MatmulPerfMode.DoubleRow
            # out rows NB*SB*q .. +NB*SB are contiguous in DRAM and match
            # the stage batch's element order exactly: one accum DMA per
            # SB groups.
            out_flat = out.rearrange("(q x) d -> q (x d)", x=NB * SB)

            def scatter_accum(q, stage4):
                nc.gpsimd.dma_start(
                    out=out_flat[q : q + 1],
                    in_=stage4[0:1],
                    accum_op=OP.add,
                )

            stage4 = None
            pending = []  # (q, stage4) scatters held until base rows land
            for g in range(G):
                xt = spool.tile([128, TI, NB, D], FP8, tag="xt")
                # alternate the stream between the two HWDGE rings (SP and
                # ACT): while one ring's tile drains, the other's
                # descriptors are already doorbelled, hiding the per-DMA
                # dispatch/doorbell bubble (~0.25us/tile on one ring).
                # g0/g1 both ride sync: the scalar ring is still loading
                # the 1.25 MiB const pack during the first ~9us.
                ring = nc.scalar if (g >= 2 and g % 2 == 0) else nc.sync
                if g < 2:
                    # quarter loads so the first matmul starts ~3.5us in
                    for u in range(TI // 2):
                        ring.dma_start(
                            out=xt[:, 2 * u : 2 * u + 2], in_=xh[:, g, 2 * u : 2 * u + 2]
                        )
                else:
                    ring.dma_start(out=xt[:], in_=xh[:, g])
                if g % SB == 0 and g < G - SB:
                    stage4 = gpool.tile([1, SB * NB * D], F32, tag="stage")
                    soff = 0
                elif g in (G - SB, G - 2):
                    # the last batch is split into two [1, 2048] tiles so
                    # the final accum RMW is half-size and starts 2 groups
                    # early without a W-after-R hazard on a shared tile
                    stage4 = gpool.tile([1, 2 * NB * D], F32, tag="stage")
                    soff = 0
                else:
                    soff += NB * D
                for cb in range(2):
                    ps = psf.tile([1, 512], F32, tag="psf")
                    for u in range(TI // 2):
                        nc.tensor.matmul(
                            ps[:],
                            c8_sb[:, 2 * u : 2 * u + 2, 0:1],
                            xt[:, 2 * u : 2 * u + 2, 4 * cb : 4 * cb + 4, :],
                            start=(u == 0),
                            stop=(u == TI // 2 - 1),
                            perf_mode=DR,
                        )
                    # drains stay off the ACT queue so the scalar ring's
                    # stream dispatches never wait behind ACT compute
                    nc.vector.tensor_scalar(
                        out=stage4[0:1, soff + cb * 512 : soff + (cb + 1) * 512],
                        in0=ps[:],
                        scalar1=SCL_OUT,
                        scalar2=None,
                        op0=OP.mult,
                    )
                if g in mlp_stages:
                    mlp_stages[g]()
                if g == 5:
                    # base rows -> out, then release the held scatter
                    for tb in range(2):
                        nc.scalar.dma_start(
                            out=out[tb * 128 : (tb + 1) * 128, :],
                            in_=base_sb[tb][:],
                        )
                    for qp, sp in pending:
                        scatter_accum(qp, sp)
                    pending.clear()
                if g in (G - 3, G - 1):
                    # half-batch accum for the tail
                    h = (g - (G - SB)) // 2
                    nc.gpsimd.dma_start(
                        out=out_flat[
                            G // SB - 1 : G // SB,
                            h * 2 * NB * D : (h + 1) * 2 * NB * D,
                        ],
                        in_=stage4[0:1],
                        accum_op=OP.add,
                    )
                elif g % SB == SB - 1 and g < G - SB:
                    q = g // SB
                    if g < 5:
                        pending.append((q, stage4))
                    else:
                        scatter_accum(q, stage4)

    nc.compile()
    return nc


_NC_CACHE = None


def _get_program() -> bass.Bass:
    global _NC_CACHE
    if _NC_CACHE is None:
        _NC_CACHE = _build_program()
    return _NC_CACHE


def _pack_consts(inputs: dict, xc: np.ndarray, nz: np.ndarray, core: int) -> np.ndarray:
    pk = np.zeros((128, NCOL), dtype=np.float32)

    def put(nm, arr):
        off, w = COLS[nm]
        pk[:, off : off + w] = arr.reshape(128, w)

    s = slice(core * B_PER, (core + 1) * B_PER)
    xcc, nzc = xc[s], nz[s]
    put("ident", np.eye(128, dtype=np.float32))
    put("xc0", xcc[0:128])
    put("xc1", xcc[128:256])
    put("nz0", nzc[0:128])
    put("nz1", nzc[128:256])
    for net in ("d", "g"):
        put(net + "w1", inputs[net + "w1"])
        w2 = inputs[net + "w2"]
        put(net + "w2_0", w2[0:128])
        put(net + "w2_1", w2[128:256])
        w3 = inputs[net + "w3"]
        put(net + "w3_0", w3[0:128])
        put(net + "w3_1", w3[128:256])
        b1 = inputs[net + "b1"]
        put(net + "b1_0", b1[0:128])
        put(net + "b1_1", b1[128:256])
        b2 = inputs[net + "b2"]
        put(net + "b2_0", b2[0:128])
        put(net + "b2_1", b2[128:256])
        put(net + "b3", inputs[net + "b3"])
    return pk


def _in_maps(inputs: dict) -> list[dict]:
    f = lambda x: np.ascontiguousarray(np.asarray(x, dtype=np.float32))
    xh = np.asarray(inputs["x_history"], dtype=np.float32)
    xc = f(inputs["x_current"])
    nz = f(inputs["noise"])
    assert xh.shape == (B, K, D) and xc.shape == (B, D) and nz.shape == (B,)
    # [core, g, bi, p, ti, d] -> [core, p, g, ti, bi, d], cast to fp8 e4m3
    xh8 = (
        xh.reshape(N_CORES, G, NB, 128, TI, D)
        .transpose(0, 3, 1, 4, 2, 5)
        .astype(E4M3)
    )
    ws = {k: f(inputs[k]) for k in inputs if k[0] in "dg" and k != "noise"}
    maps = []
    for c in range(N_CORES):
        maps.append({"xh": xh8[c], "wp": _pack_consts(ws, xc, nz, c)})
    return maps


def run(inputs: dict, trace: bool = False):
    nc = _get_program()
    res = run_bass_kernel_spmd(nc, _in_maps(inputs), list(range(N_CORES)), trace=trace)
    out = np.concatenate([res.results[c]["out"] for c in range(N_CORES)], axis=0)
    return out, res


def kernel(**inputs) -> np.ndarray:
    out, _ = run(inputs, trace=False)
    return out


# revision 19
# speedup vs baseline: 1.0446x; 1.0274x over previous
"""Trainium2 Bass kernel for NeuralFractionalDE.

out = x_current + drift(x)*DT + softplus_head(x)*(noise*DT^H) + frac_deriv*(ALPHA*DT)

where frac_deriv = sum_k (x_hist[:,k+1,:]-x_hist[:,k,:]) * w[k] collapses to
sum_t c[t] * x_hist[:,t,:] with c[t] = w[t-1]-w[t] (boundary adjusted).

Data parallel over 8 NeuronCores (256 batch rows each). The x_history
stream is cast to fp8 e4m3 on the host (error contribution ~5e-5 rel-fro,
two orders under the gate) and rearranged host-side to a partition-major
layout: t = 8*p + ti, so each partition's whole stream is contiguous in
HBM (one 8 KiB descriptor per partition per group). The time reduction
runs on the TensorEngine as DoubleRow fp8 matmuls: each instruction
contracts 2 timesteps (128 partitions x 2), 4 accumulating matmuls per
512-wide PSUM row. Coefficients are scaled by 64 into fp8 range; the
ALPHA*DT/64 factor is applied in the PSUM copy-out.

Output assembly: base rows (xc + drift*DT + diffusion*fbm) are written to
DRAM `out` once, then the stream results are CCE-accumulated into `out`
by SWDGE scatter DMAs, batched 4 groups per descriptor chain (per-group
4 KiB accum DMAs serialize at ~2.8us each on the Q7 + completion path
and would lag the stream). All fp32 constants arrive via one packed
[128, NCOL] tensor: per-tensor const loads cost ~0.7us of HWDGE ring
dispatch each and delayed the MLP by ~20us.
"""

import math

import numpy as np

try:
    import concourse.bass as bass
except ImportError:  # pragma: no cover
    import sys

    sys.path.insert(0, "/opt/trn_rl_repo")
    import concourse.bass as bass

import ml_dtypes

import concourse.bacc as bacc
import concourse.mybir as mybir
import concourse.tile as tile
from concourse.bass_utils import run_bass_kernel_spmd

ALPHA = 0.7
K = 1024
DT = 0.01
H = 0.5 + ALPHA / 2
D = 128
HID = 256
B = 2048
N_CORES = 8
B_PER = B // N_CORES  # 256
TI = 8  # time sub-steps per partition: t = TI*p + ti
NB = 8  # batch rows per group
G = B_PER // NB  # 32 groups; batch b = NB*g + bi
SB = 4  # groups per scatter-accum batch
CSCALE = 64.0  # fp8 range scale for the frac coefficients
SCL_OUT = float(ALPHA * DT / CSCALE)

F32 = mybir.dt.float32
FP8 = mybir.dt.float8e4
AF = mybir.ActivationFunctionType
OP = mybir.AluOpType
E4M3 = ml_dtypes.float8_e4m3


def _pack_spec():
    cols = {}
    off = 0

    def add(name, w):
        nonlocal off
        cols[name] = (off, w)
        off += w

    add("ident", 128)
    add("xc0", 128)
    add("xc1", 128)
    add("nz0", 1)
    add("nz1", 1)
    for net in ("d", "g"):
        add(net + "w1", HID)
        add(net + "w2_0", HID)
        add(net + "w2_1", HID)
        add(net + "w3_0", D)
        add(net + "w3_1", D)
        add(net + "b1_0", 1)
        add(net + "b1_1", 1)
        add(net + "b2_0", 1)
        add(net + "b2_1", 1)
        add(net + "b3", 1)
    return cols, off


COLS, NCOL = _pack_spec()


def _coeffs_fp8() -> np.ndarray:
    """c8st[p, ti, 0] = c[TI*p + ti] * CSCALE in fp8; Ko-stride 16 B."""
    t = np.arange(1, K + 1, dtype=np.float64)
    kern = (t ** (-ALPHA)) / math.gamma(1.0 - ALPHA)
    w = kern[::-1][: K - 1]  # w[k] = kern[K-1-k]
    c = np.zeros(K, dtype=np.float64)
    c[1:] += w
    c[: K - 1] -= w
    c *= CSCALE
    arr = np.zeros((128, TI, 16), dtype=E4M3)
    arr[:, :, 0] = c.reshape(128, TI).astype(np.float32).astype(E4M3)
    return arr


def _build_program() -> bass.Bass:
    # Bacc (not raw Bass): its compile() legalizes semaphore waits to the
    # 1-wait-per-instruction ISA limit (generate_event_semaphores).
    nc = bacc.Bacc(None, target_bir_lowering=False)

    xh = nc.dram_tensor("xh", [128, G, TI, NB, D], FP8, kind="ExternalInput")
    wp = nc.dram_tensor("wp", [128, NCOL], F32, kind="ExternalInput")
    out = nc.dram_tensor("out", [B_PER, D], F32, kind="ExternalOutput")

    c8d = nc.inline_tensor(_coeffs_fp8(), name="c8const")

    with tile.TileContext(nc) as tc:
        with (
            tc.tile_pool(name="const", bufs=1) as cpool,
            tc.tile_pool(name="stream", bufs=8) as spool,
            tc.tile_pool(name="stg", bufs=4) as gpool,
            tc.tile_pool(name="psf", bufs=4, space=bass.MemorySpace.PSUM) as psf,
            tc.tile_pool(name="psm", bufs=2, space=bass.MemorySpace.PSUM) as psm,
            tc.tile_pool(name="pst", bufs=2, space=bass.MemorySpace.PSUM) as pst,
        ):
            # ---- constants: one fp8 coeff load + one packed fp32 load on
            # the scalar (ACT) HWDGE ring; the sync ring is reserved for
            # the x_history stream ----
            c8_sb = cpool.tile([128, TI, 16], FP8, tag="c8")
            nc.scalar.dma_start(out=c8_sb[:], in_=c8d[:])
            wp_sb = cpool.tile([128, NCOL], F32, tag="wp")
            nc.scalar.dma_start(out=wp_sb[:], in_=wp[:])

            def C(nm, c0=0, w=None):
                off, width = COLS[nm]
                if w is None:
                    w = width
                return wp_sb[:, off + c0 : off + c0 + w]

            base_sb = [
                cpool.tile([128, D], F32, tag=f"base{tb}", name=f"base{tb}")
                for tb in range(2)
            ]
            xcT_sb = cpool.tile([128, B_PER], F32, tag="xcT")
            driftT_sb = cpool.tile([128, B_PER], F32, tag="driftT")
            diffT_sb = cpool.tile([128, B_PER], F32, tag="diffT")

            # ACT LUT discipline: tanh and exp live in the same table set
            # (exp_and_others), ln in another; each ACT_TABLE_LOAD costs
            # ~1.3us on the ACT queue that also drains stream PSUMs. Using
            # AF.Tanh directly + one Exp->Ln switch for the softplus head
            # keeps it to a single mid-kernel table load. (Copy is a
            # size-1 "others" entry present in every table.)

            h_sb = {}  # MLP hidden tiles, created per stage

            # ---- MLP emitted piecewise between stream groups so the PE
            # queue never stalls on ACT/DVE latency ----
            def mlp_stage_xcT():
                for tb in range(2):
                    pt = pst.tile([128, 128], F32, tag="pst")
                    nc.tensor.transpose(pt[:], C(f"xc{tb}"), C("ident"))
                    nc.scalar.activation(
                        xcT_sb[:, tb * 128 : (tb + 1) * 128], pt[:], AF.Copy
                    )

            def mlp_stage_h1():
                for net in ("d", "g"):
                    h1 = []
                    for j in range(2):
                        ps = psm.tile([128, B_PER], F32, tag="psm")
                        nc.tensor.matmul(
                            ps[:],
                            C(net + "w1", j * 128, 128),
                            xcT_sb[:],
                            start=True,
                            stop=True,
                        )
                        h = cpool.tile([128, B_PER], F32, tag=f"{net}h1{j}")
                        nc.scalar.activation(
                            h[:], ps[:], AF.Tanh, bias=C(net + f"b1_{j}")
                        )
                        h1.append(h)
                    h_sb[net + "h1"] = h1

            def mlp_stage_h2():
                for net in ("d", "g"):
                    h1 = h_sb[net + "h1"]
                    h2 = []
                    for j in range(2):
                        ps = psm.tile([128, B_PER], F32, tag="psm")
                        for i in range(2):
                            nc.tensor.matmul(
                                ps[:],
                                C(net + f"w2_{i}", j * 128, 128),
                                h1[i][:],
                                start=(i == 0),
                                stop=(i == 1),
                            )
                        h = cpool.tile([128, B_PER], F32, tag=f"{net}h2{j}")
                        nc.scalar.activation(
                            h[:], ps[:], AF.Tanh, bias=C(net + f"b2_{j}")
                        )
                        h2.append(h)
                    h_sb[net + "h2"] = h2

            def mlp_stage_out():
                for net in ("d", "g"):
                    h2 = h_sb[net + "h2"]
                    ps = psm.tile([128, B_PER], F32, tag="psm")
                    for i in range(2):
                        nc.tensor.matmul(
                            ps[:],
                            C(net + f"w3_{i}"),
                            h2[i][:],
                            start=(i == 0),
                            stop=(i == 1),
                        )
                    if net == "d":
                        # driftT = (raw + b3) * DT
                        nc.vector.tensor_scalar(
                            out=driftT_sb[:],
                            in0=ps[:],
                            scalar1=C("db3"),
                            scalar2=float(DT),
                            op0=OP.add,
                            op1=OP.mult,
                        )
                    else:
                        # softplus via ln(1 + exp(x + b)); the Ln is the
                        # kernel's single ACT table switch
                        nc.scalar.activation(
                            diffT_sb[:], ps[:], AF.Exp, bias=C("gb3")
                        )
                        nc.scalar.activation(diffT_sb[:], diffT_sb[:], AF.Ln, bias=1.0)

            def mlp_stage_base():
                # base[tb] = xc + driftT^T (already *DT) + diffT^T*nz*DT^H
                for tb in range(2):
                    ptd = pst.tile([128, 128], F32, tag="pst")
                    nc.tensor.transpose(
                        ptd[:], driftT_sb[:, tb * 128 : (tb + 1) * 128], C("ident")
                    )
                    ptg = pst.tile([128, 128], F32, tag="pst")
                    nc.tensor.transpose(
                        ptg[:], diffT_sb[:, tb * 128 : (tb + 1) * 128], C("ident")
                    )
                    b_ = base_sb[tb]
                    # base = diffusion * noise * DT^H
                    nc.vector.tensor_scalar(
                        out=b_[:],
                        in0=ptg[:],
                        scalar1=C(f"nz{tb}"),
                        scalar2=float(DT**H),
                        op0=OP.mult,
                        op1=OP.mult,
                    )
                    nc.vector.tensor_add(out=b_[:], in0=b_[:], in1=ptd[:])
                    nc.vector.tensor_add(out=b_[:], in0=b_[:], in1=C(f"xc{tb}"))

            mlp_stages = {
                1: mlp_stage_xcT,
                2: mlp_stage_h1,
                3: mlp_stage_h2,
                4: mlp_stage_out,
                5: mlp_stage_base,
            }

            # ---- fractional-derivative stream: the 32 MiB fp8 scan ----
            # xh[p, g, ti, bi, d]: per partition, one 8 KiB contiguous read
            # per group. DoubleRow contracts timestep pairs (2u, 2u+1):
            # lhsT = c8[:, 2u:2u+2, 0:1] (Ko stride 16 B), rhs free = 1024
            # -> psum [1, 512] over 4 accumulating matmuls.
            DR = mybir.MatmulPerfMode.DoubleRow
            # out rows NB*SB*q .. +NB*SB are contiguous in DRAM and match
            # the stage batch's element order exactly: one accum DMA per
            # SB groups.
            out_flat = out.rearrange("(q x) d -> q (x d)", x=NB * SB)

            def scatter_accum(q, stage4):
                nc.gpsimd.dma_start(
                    out=out_flat[q : q + 1],
                    in_=stage4[0:1],
                    accum_op=OP.add,
                )

            stage4 = None
            for g in range(G):
                xt = spool.tile([128, TI, NB, D], FP8, tag="xt")
                # alternate the stream between the two HWDGE rings (SP and
                # ACT): while one ring's tile drains, the other's
                # descriptors are already doorbelled, hiding the per-DMA
                # dispatch/doorbell bubble (~0.25us/tile on one ring).
                # g0/g1 both ride sync: the scalar ring is still loading
                # the 1.25 MiB const pack during the first ~9us.
                ring = nc.scalar if (g >= 2 and g % 2 == 0) else nc.sync
                if g < 2:
                    # quarter loads so the first matmul starts ~3.5us in
                    for u in range(TI // 2):
                        ring.dma_start(
                            out=xt[:, 2 * u : 2 * u + 2], in_=xh[:, g, 2 * u : 2 * u + 2]
                        )
                else:
                    ring.dma_start(out=xt[:], in_=xh[:, g])
                if g % SB == 0 and g < G - SB:
                    stage4 = gpool.tile([1, SB * NB * D], F32, tag="stage")
                    soff = 0
                elif g in (G - SB, G - 2):
                    # the last batch is split into two [1, 2048] tiles so
                    # the final accum RMW is half-size and starts 2 groups
                    # early without a W-after-R hazard on a shared tile
                    stage4 = gpool.tile([1, 2 * NB * D], F32, tag="stage")
                    soff = 0
                else:
                    soff += NB * D
                for cb in range(2):
                    ps = psf.tile([1, 512], F32, tag="psf")
                    for u in range(TI // 2):
                        nc.tensor.matmul(
                            ps[:],
                            c8_sb[:, 2 * u : 2 * u + 2, 0:1],
                            xt[:, 2 * u : 2 * u + 2, 4 * cb : 4 * cb + 4, :],
                            start=(u == 0),
                            stop=(u == TI // 2 - 1),
                            perf_mode=DR,
                        )
                    # drains stay off the ACT queue so the scalar ring's
                    # stream dispatches never wait behind ACT compute
                    nc.vector.tensor_scalar(
                        out=stage4[0:1, soff + cb * 512 : soff + (cb + 1) * 512],
                        in0=ps[:],
                        scalar1=SCL_OUT,
                        scalar2=None,
                        op0=OP.mult,
                    )
                if g in mlp_stages:
                    mlp_stages[g]()
                if g == 24:
                    # PJRT/run_neff pre-zero ExternalOutput buffers and
                    # adds commute, so base rows are CCE-accumulated like
                    # the frac rows -- scatters never wait on the (lazily
                    # trailing) MLP chain. Emitted at g=24 so the in-order
                    # gpsimd queue is past the data-ready scatters by then.
                    for tb in range(2):
                        nc.gpsimd.dma_start(
                            out=out[tb * 128 : (tb + 1) * 128, :],
                            in_=base_sb[tb][:],
                            accum_op=OP.add,
                        )
                if g in (G - 3, G - 1):
                    # half-batch accum for the tail
                    h = (g - (G - SB)) // 2
                    nc.gpsimd.dma_start(
                        out=out_flat[
                            G // SB - 1 : G // SB,
                            h * 2 * NB * D : (h + 1) * 2 * NB * D,
                        ],
                        in_=stage4[0:1],
                        accum_op=OP.add,
                    )
                elif g % SB == SB - 1 and g < G - SB:
                    scatter_accum(g // SB, stage4)

    nc.compile()
    return nc


_NC_CACHE = None


def _get_program() -> bass.Bass:
    global _NC_CACHE
    if _NC_CACHE is None:
        _NC_CACHE = _build_program()
    return _NC_CACHE


def _pack_consts(inputs: dict, xc: np.ndarray, nz: np.ndarray, core: int) -> np.ndarray:
    pk = np.zeros((128, NCOL), dtype=np.float32)

    def put(nm, arr):
        off, w = COLS[nm]
        pk[:, off : off + w] = arr.reshape(128, w)

    s = slice(core * B_PER, (core + 1) * B_PER)
    xcc, nzc = xc[s], nz[s]
    put("ident", np.eye(128, dtype=np.float32))
    put("xc0", xcc[0:128])
    put("xc1", xcc[128:256])
    put("nz0", nzc[0:128])
    put("nz1", nzc[128:256])
    for net in ("d", "g"):
        put(net + "w1", inputs[net + "w1"])
        w2 = inputs[net + "w2"]
        put(net + "w2_0", w2[0:128])
        put(net + "w2_1", w2[128:256])
        w3 = inputs[net + "w3"]
        put(net + "w3_0", w3[0:128])
        put(net + "w3_1", w3[128:256])
        b1 = inputs[net + "b1"]
        put(net + "b1_0", b1[0:128])
        put(net + "b1_1", b1[128:256])
        b2 = inputs[net + "b2"]
        put(net + "b2_0", b2[0:128])
        put(net + "b2_1", b2[128:256])
        put(net + "b3", inputs[net + "b3"])
    return pk


def _in_maps(inputs: dict) -> list[dict]:
    f = lambda x: np.ascontiguousarray(np.asarray(x, dtype=np.float32))
    xh = np.asarray(inputs["x_history"], dtype=np.float32)
    xc = f(inputs["x_current"])
    nz = f(inputs["noise"])
    assert xh.shape == (B, K, D) and xc.shape == (B, D) and nz.shape == (B,)
    # [core, g, bi, p, ti, d] -> [core, p, g, ti, bi, d], cast to fp8 e4m3
    xh8 = (
        xh.reshape(N_CORES, G, NB, 128, TI, D)
        .transpose(0, 3, 1, 4, 2, 5)
        .astype(E4M3)
    )
    ws = {k: f(inputs[k]) for k in inputs if k[0] in "dg" and k != "noise"}
    maps = []
    for c in range(N_CORES):
        maps.append({"xh": xh8[c], "wp": _pack_consts(ws, xc, nz, c)})
    return maps


def run(inputs: dict, trace: bool = False):
    nc = _get_program()
    res = run_bass_kernel_spmd(nc, _in_maps(inputs), list(range(N_CORES)), trace=trace)
    out = np.concatenate([res.results[c]["out"] for c in range(N_CORES)], axis=0)
    return out, res


def kernel(**inputs) -> np.ndarray:
    out, _ = run(inputs, trace=False)
    return out
